# revision 2
# baseline (speedup 1.0000x reference)
"""Trainium2 Bass kernel for AttentionModule (channel attention over 64x64 images).

Computation (per batch b):
  qkv = qkv_w @ x + qkv_b            (1x1 conv, x viewed as [C, N=H*W])
  q,k,v: [heads=8, d=64, N]
  S_h   = q_h @ k_h^T * d^-0.5       ([64, 64] per head -- contraction over N!)
  attn  = softmax(S, axis=-1)
  out   = attn @ v -> [C, N]
  out   = proj_w @ out + proj_b
  returns (out [B,C,H,W], attn.mean(heads) [B,64,64])

Sharding: data-parallel over batch, 1 batch per NeuronCore, weights replicated.

Per-core dataflow (all matmul inputs fp16, fp32 PSUM accumulation):
  - q,k computed pixel-major: lhsT = x chunk [c,128n] (stationary),
    rhs = Wqk^T [c, o] (moving) -> psum [128n, 512o]; scores accumulate
    S[64d, 8h, 64d] over 32 n-chunks in a dedicated PSUM bank.
  - softmax on ACT/DVE (exp via activation with per-partition bias = -max*scale).
  - v / attn@v / proj computed channel-major; attn transposed on PE and packed
    block-diagonally (2 heads per [128,128] lhsT).
"""

import sys

for _p in ("/root/.axon_site/_ro/trn_rl_repo", "/opt/trn_rl_repo"):
    if _p not in sys.path:
        sys.path.append(_p)

import numpy as np

import concourse.bacc as bacc
import concourse.bass as bass
import concourse.mybir as mybir
import concourse.tile as tile
from concourse.bass_utils import run_bass_kernel_spmd
from concourse.masks import make_identity

F16 = mybir.dt.float16
F32 = mybir.dt.float32
AF = mybir.ActivationFunctionType
AX = mybir.AxisListType

C = 512          # channels
N = 4096         # pixels (64*64)
HEADS = 8
D = 64           # head dim
CCH = C // 128   # channel chunks of 128 partitions
SCALE = float(D) ** -0.5


def build_nc():
    nc = bacc.Bacc("TRN2", target_bir_lowering=False, debug=False)

    x_d = nc.dram_tensor("x", [C, N], F16, kind="ExternalInput")
    wqk_d = nc.dram_tensor("wqk", [C, 2 * C], F16, kind="ExternalInput")  # qkv_w[:2C].T
    wv_d = nc.dram_tensor("wv", [C, C], F16, kind="ExternalInput")        # qkv_w[2C:].T
    wp_d = nc.dram_tensor("wp", [C, C], F16, kind="ExternalInput")        # proj_w.T
    bqk_d = nc.dram_tensor("bqk", [2 * C], F32, kind="ExternalInput")
    bv_d = nc.dram_tensor("bv", [C], F32, kind="ExternalInput")
    bp_d = nc.dram_tensor("bp", [C], F32, kind="ExternalInput")
    out_d = nc.dram_tensor("out", [C, N], F32, kind="ExternalOutput")
    am_d = nc.dram_tensor("attn_mean", [D, D], F32, kind="ExternalOutput")

    with tile.TileContext(nc) as tc:
        with (
            tc.tile_pool(name="consts", bufs=1) as consts,
            tc.tile_pool(name="work", bufs=4) as work,
            tc.tile_pool(name="small", bufs=1) as small,
            tc.tile_pool(name="ypool", bufs=4) as ypool,
            tc.tile_pool(name="mmpsum", bufs=6, space="PSUM") as mmpsum,
            tc.tile_pool(name="spsum", bufs=1, space="PSUM") as spsum,
        ):
            # ---- load inputs to SBUF ----
            x_sb = []
            for c in range(CCH):
                t = consts.tile([128, N], F16, tag=f"x{c}")
                nc.sync.dma_start(out=t, in_=x_d[c * 128:(c + 1) * 128, :])
                x_sb.append(t)

            wqk_sb = consts.tile([128, CCH, 2 * C], F16, tag="wqk")
            nc.sync.dma_start(out=wqk_sb, in_=wqk_d.rearrange("(cc p) o -> p cc o", p=128))
            wv_sb = consts.tile([128, CCH, C], F16, tag="wv")
            nc.sync.dma_start(out=wv_sb, in_=wv_d.rearrange("(cc p) o -> p cc o", p=128))
            wp_sb = consts.tile([128, CCH, C], F16, tag="wp")
            nc.sync.dma_start(out=wp_sb, in_=wp_d.rearrange("(cc p) o -> p cc o", p=128))

            # q,k bias varies along the free dim -> broadcast over partitions via DMA
            bqk_sb = consts.tile([128, 2 * C], F32, tag="bqk")
            bqk_ap = bqk_d.ap()
            bqk_bcast = bass.AP(tensor=bqk_ap.tensor, offset=0,
                                ap=[[0, 128]] + list(bqk_ap.ap))
            nc.gpsimd.dma_start(out=bqk_sb, in_=bqk_bcast)
            # v,proj bias: per-partition [128, chunk]
            bv_sb = consts.tile([128, CCH], F32, tag="bv")
            nc.sync.dma_start(out=bv_sb, in_=bv_d.rearrange("(o p) -> p o", p=128))
            bp_sb = consts.tile([128, CCH], F32, tag="bp")
            nc.sync.dma_start(out=bp_sb, in_=bp_d.rearrange("(o p) -> p o", p=128))

            ident = consts.tile([D, D], F16, tag="ident")
            make_identity(nc, ident)

            # ---- phase 1: q,k pixel-major + score accumulation ----
            # S[dq 64, h 8, dk 64] accumulates in one PSUM bank over all 32 n-chunks.
            s_psum = spsum.tile([D, HEADS, D], F32, tag="s")
            NT1 = N // 128
            for n in range(NT1):
                pq = mmpsum.tile([128, 512], F32, tag="mm")
                pk = mmpsum.tile([128, 512], F32, tag="mm")
                for c in range(CCH):
                    nc.tensor.matmul(pq, lhsT=x_sb[c][:, n * 128:(n + 1) * 128],
                                     rhs=wqk_sb[:, c, 0:512],
                                     start=(c == 0), stop=(c == CCH - 1))
                for c in range(CCH):
                    nc.tensor.matmul(pk, lhsT=x_sb[c][:, n * 128:(n + 1) * 128],
                                     rhs=wqk_sb[:, c, 512:1024],
                                     start=(c == 0), stop=(c == CCH - 1))
                qt = work.tile([128, 512], F16, tag="qt")
                kt = work.tile([128, 512], F16, tag="kt")
                nc.vector.tensor_add(out=qt, in0=pq, in1=bqk_sb[:, 0:512])
                nc.vector.tensor_add(out=kt, in0=pk, in1=bqk_sb[:, 512:1024])
                for h in range(HEADS):
                    nc.tensor.matmul(s_psum[:, h, :],
                                     lhsT=qt[:, h * D:(h + 1) * D],
                                     rhs=kt[:, h * D:(h + 1) * D],
                                     start=(n == 0 and h == 0), stop=(n == NT1 - 1),
                                     skip_group_check=True)

            # ---- phase 2: v channel-major ----
            v_sb = []
            for o in range(CCH):
                t = consts.tile([128, N], F16, tag=f"v{o}")
                v_sb.append(t)
            NT2 = N // 512
            for o in range(CCH):
                for n in range(NT2):
                    pv = mmpsum.tile([128, 512], F32, tag="mm")
                    for c in range(CCH):
                        nc.tensor.matmul(pv, lhsT=wv_sb[:, c, o * 128:(o + 1) * 128],
                                         rhs=x_sb[c][:, n * 512:(n + 1) * 512],
                                         start=(c == 0), stop=(c == CCH - 1))
                    nc.scalar.activation(v_sb[o][:, n * 512:(n + 1) * 512], pv,
                                         AF.Identity, bias=bv_sb[:, o:o + 1], scale=1.0)

            # ---- phase 3: softmax over dk ----
            mx = small.tile([D, HEADS], F32, tag="mx")
            nc.vector.reduce_max(out=mx, in_=s_psum, axis=AX.X)
            nmx = small.tile([D, HEADS], F32, tag="nmx")
            nc.vector.tensor_scalar_mul(nmx, mx, -SCALE)
            e_sb = small.tile([D, HEADS, D], F32, tag="e")
            for h in range(HEADS):
                nc.scalar.activation(e_sb[:, h, :], s_psum[:, h, :], AF.Exp,
                                     bias=nmx[:, h:h + 1], scale=SCALE)
            sm = small.tile([D, HEADS], F32, tag="sm")
            nc.vector.reduce_sum(out=sm, in_=e_sb, axis=AX.X)
            rs = small.tile([D, HEADS], F32, tag="rs")
            nc.vector.reciprocal(rs, sm)
            attn32 = small.tile([D, HEADS, D], F32, tag="attn32")
            attn16 = small.tile([D, HEADS, D], F16, tag="attn16")
            for h in range(HEADS):
                nc.vector.tensor_scalar_mul(attn32[:, h, :], e_sb[:, h, :], rs[:, h:h + 1])
            nc.vector.tensor_copy(out=attn16, in_=attn32)

            # attn_mean over heads: reduce the (h) axis by viewing [d, dk, h]
            am_sb = small.tile([D, D], F32, tag="am")
            attn_hview = attn32.rearrange("p h d -> p d h")
            nc.vector.reduce_sum(out=am_sb, in_=attn_hview, axis=AX.X)
            nc.vector.tensor_scalar_mul(am_sb, am_sb, 1.0 / HEADS)
            nc.sync.dma_start(out=am_d[:, :], in_=am_sb)

            # ---- phase 4: attn^T block-diag pack (2 heads per [128,128] lhsT) ----
            avl = []
            for g in range(CCH):
                t = consts.tile([128, 128], F16, tag=f"avl{g}")
                nc.vector.memset(t, 0.0)
                avl.append(t)
            for g in range(CCH):
                pt = mmpsum.tile([128, 128], F16, tag="mm")
                for r in range(2):
                    h = 2 * g + r
                    nc.tensor.transpose(pt[r * D:(r + 1) * D, r * D:(r + 1) * D],
                                        attn16[:, h, :], ident)
                for r in range(2):
                    nc.vector.tensor_copy(out=avl[g][r * D:(r + 1) * D, r * D:(r + 1) * D],
                                          in_=pt[r * D:(r + 1) * D, r * D:(r + 1) * D])

            # ---- phase 5: av = attn @ v (channel-major out) ----
            av_sb = []
            for g in range(CCH):
                t = consts.tile([128, N], F16, tag=f"av{g}")
                av_sb.append(t)
            for g in range(CCH):
                for n in range(NT2):
                    pav = mmpsum.tile([128, 512], F32, tag="mm")
                    nc.tensor.matmul(pav, lhsT=avl[g], rhs=v_sb[g][:, n * 512:(n + 1) * 512],
                                     start=True, stop=True)
                    nc.scalar.activation(av_sb[g][:, n * 512:(n + 1) * 512], pav,
                                         AF.Identity, bias=0.0, scale=1.0)

            # ---- phase 6: proj ----
            for n in range(NT2):
                for o in range(CCH):
                    py = mmpsum.tile([128, 512], F32, tag="mm")
                    for c in range(CCH):
                        nc.tensor.matmul(py, lhsT=wp_sb[:, c, o * 128:(o + 1) * 128],
                                         rhs=av_sb[c][:, n * 512:(n + 1) * 512],
                                         start=(c == 0), stop=(c == CCH - 1))
                    y = ypool.tile([128, 512], F32, tag="y")
                    nc.scalar.activation(y, py, AF.Identity, bias=bp_sb[:, o:o + 1], scale=1.0)
                    nc.sync.dma_start(out=out_d[o * 128:(o + 1) * 128, n * 512:(n + 1) * 512],
                                      in_=y)

    nc.compile()
    return nc


_NC_CACHE = None


def _get_nc():
    global _NC_CACHE
    if _NC_CACHE is None:
        _NC_CACHE = build_nc()
    return _NC_CACHE


def make_in_maps(x, qkv_w, qkv_b, proj_w, proj_b):
    x = np.asarray(x, dtype=np.float32)
    qkv_w = np.asarray(qkv_w, dtype=np.float32)
    qkv_b = np.asarray(qkv_b, dtype=np.float32)
    proj_w = np.asarray(proj_w, dtype=np.float32)
    proj_b = np.asarray(proj_b, dtype=np.float32)
    B = x.shape[0]
    xm = x.reshape(B, C, N).astype(np.float16)
    wqk = np.ascontiguousarray(qkv_w[:2 * C].T).astype(np.float16)
    wv = np.ascontiguousarray(qkv_w[2 * C:].T).astype(np.float16)
    wp = np.ascontiguousarray(proj_w.T).astype(np.float16)
    bqk = np.ascontiguousarray(qkv_b[:2 * C])
    bv = np.ascontiguousarray(qkv_b[2 * C:])
    bp = np.ascontiguousarray(proj_b)
    return [
        {"x": xm[b], "wqk": wqk, "wv": wv, "wp": wp, "bqk": bqk, "bv": bv, "bp": bp}
        for b in range(B)
    ]


def kernel(x, qkv_w, qkv_b, proj_w, proj_b):
    B, _, H, W = np.asarray(x).shape
    in_maps = make_in_maps(x, qkv_w, qkv_b, proj_w, proj_b)
    nc = _get_nc()
    res = run_bass_kernel_spmd(nc, in_maps, core_ids=list(range(B)))
    out = np.stack([res.results[b]["out"] for b in range(B)]).reshape(B, C, H, W)
    am = np.stack([res.results[b]["attn_mean"] for b in range(B)])
    return out.astype(np.float32), am.astype(np.float32)


if __name__ == "__main__":
    nc = build_nc()
    print("built ok")


# revision 5
# speedup vs baseline: 1.0282x; 1.0282x over previous
"""Trainium2 Bass kernel for AttentionModule (channel attention over 64x64 images).

Computation (per batch b):
  qkv = qkv_w @ x + qkv_b            (1x1 conv, x viewed as [C, N=H*W])
  q,k,v: [heads=8, d=64, N]
  S_h   = q_h @ k_h^T * d^-0.5       ([64, 64] per head -- contraction over N!)
  attn  = softmax(S, axis=-1)
  out   = attn @ v -> [C, N]
  out   = proj_w @ out + proj_b
  returns (out [B,C,H,W], attn.mean(heads) [B,64,64])

Sharding: data-parallel over batch, 1 batch per NeuronCore, weights replicated.

Per-core dataflow (fp16 matmul inputs, fp32 PSUM accumulation):
  Scores use the Gram-matrix identity (cheaper than computing pixel-major q,k):
    S_h = Wq_h G Wk_h^T + (Wq_h s + N bq_h) bk_h^T + bq_h (Wk_h s)^T,
    G = x x^T, s = x @ 1.
  - x arrives in 32 column-block tiles; each is PE-transposed; G accumulates
    in 4 PSUM banks while x still streams in.
  - T = G @ Wk^T, then per-head S = Wq_h @ T_h plus rank-1 bias corrections.
  - softmax on DVE/ACT; v / attn@v / proj channel-major; attn^T packed
    block-diagonally (2 heads per [128,128] lhsT).
"""

import sys

for _p in ("/root/.axon_site/_ro/trn_rl_repo", "/opt/trn_rl_repo"):
    if _p not in sys.path:
        sys.path.append(_p)

import numpy as np

import concourse.bacc as bacc
import concourse.bass as bass
import concourse.mybir as mybir
import concourse.tile as tile
from concourse.bass_utils import run_bass_kernel_spmd
from concourse.masks import make_identity

F16 = mybir.dt.float16
F32 = mybir.dt.float32
AF = mybir.ActivationFunctionType
AX = mybir.AxisListType

C = 512          # channels
N = 4096         # pixels (64*64)
HEADS = 8
D = 64           # head dim
CCH = C // 128   # channel chunks of 128 partitions
NB = N // 512    # x column blocks (512 wide)
NT = N // 128    # pixel chunks of 128 (for transposes)
SCALE = float(D) ** -0.5


def build_nc():
    nc = bacc.Bacc("TRN2", target_bir_lowering=False, debug=False)

    x_d = nc.dram_tensor("x", [C, N], F16, kind="ExternalInput")
    wqk_d = nc.dram_tensor("wqk", [C, 2 * C], F16, kind="ExternalInput")  # qkv_w[:2C].T
    wv_d = nc.dram_tensor("wv", [C, C], F16, kind="ExternalInput")        # qkv_w[2C:].T
    wp_d = nc.dram_tensor("wp", [C, C], F16, kind="ExternalInput")        # proj_w.T
    bqk_d = nc.dram_tensor("bqk", [2 * C], F32, kind="ExternalInput")
    bv_d = nc.dram_tensor("bv", [C], F32, kind="ExternalInput")
    bp_d = nc.dram_tensor("bp", [C], F32, kind="ExternalInput")
    out_d = nc.dram_tensor("out", [C, N], F32, kind="ExternalOutput")
    am_d = nc.dram_tensor("attn_mean", [D, D], F32, kind="ExternalOutput")

    with tile.TileContext(nc) as tc:
        with (
            tc.tile_pool(name="consts", bufs=1) as consts,
            tc.tile_pool(name="small", bufs=1) as small,
            tc.tile_pool(name="ypool", bufs=4) as ypool,
            tc.tile_pool(name="mmpsum", bufs=3, space="PSUM") as mmpsum,
            tc.tile_pool(name="gpsum", bufs=1, space="PSUM") as gpsum,
            tc.tile_pool(name="spsum", bufs=1, space="PSUM") as spsum,
        ):
            # ---- input DMAs (x first, in column blocks, so PE can start early) ----
            x_sb = [[None] * NB for _ in range(CCH)]
            for j in range(NB):
                for c in range(CCH):
                    t = consts.tile([128, 512], F16, tag=f"x{c}_{j}")
                    nc.sync.dma_start(out=t, in_=x_d[c * 128:(c + 1) * 128,
                                                     j * 512:(j + 1) * 512])
                    x_sb[c][j] = t

            wqk_sb = []
            for c in range(CCH):
                t = consts.tile([128, 2 * C], F16, tag=f"wqk{c}")
                nc.sync.dma_start(out=t, in_=wqk_d[c * 128:(c + 1) * 128, :])
                wqk_sb.append(t)
            wv_sb = consts.tile([128, CCH, C], F16, tag="wv")
            nc.sync.dma_start(out=wv_sb, in_=wv_d.rearrange("(cc p) o -> p cc o", p=128))
            wp_sb = consts.tile([128, CCH, C], F16, tag="wp")
            nc.sync.dma_start(out=wp_sb, in_=wp_d.rearrange("(cc p) o -> p cc o", p=128))

            bqk_row = small.tile([1, 2 * C], F32, tag="bqkrow")
            nc.sync.dma_start(out=bqk_row, in_=bqk_d[None, :])
            bv_sb = consts.tile([128, CCH], F32, tag="bv")
            nc.sync.dma_start(out=bv_sb, in_=bv_d.rearrange("(o p) -> p o", p=128))
            bp_sb = consts.tile([128, CCH], F32, tag="bp")
            nc.sync.dma_start(out=bp_sb, in_=bp_d.rearrange("(o p) -> p o", p=128))

            ident = consts.tile([128, 128], F16, tag="ident")
            make_identity(nc, ident)

            # ---- phase 1: transpose x and accumulate G = x x^T ----
            # xT[n-chunk][128 n, 512 c], G accumulates in 4 dedicated PSUM banks.
            g_ps = [gpsum.tile([128, 512], F32, tag=f"g{t}", name=f"g{t}") for t in range(CCH)]
            xT_sb = []
            for n in range(NT):
                j, s = divmod(n, 4)
                pxt = mmpsum.tile([128, 512], F16, tag="mm")
                for c in range(CCH):
                    nc.tensor.transpose(pxt[:, c * 128:(c + 1) * 128],
                                        x_sb[c][j][:, s * 128:(s + 1) * 128], ident)
                xt = consts.tile([128, 512], F16, tag=f"xt{n}")
                nc.vector.tensor_copy(out=xt, in_=pxt)
                xT_sb.append(xt)
                for t in range(CCH):
                    nc.tensor.matmul(g_ps[t], lhsT=xt[:, t * 128:(t + 1) * 128],
                                     rhs=xt[:, :],
                                     start=(n == 0), stop=(n == NT - 1))

            # row sums s = x @ 1 (gpsimd reduces; fp32)
            s_part = small.tile([128, CCH, NB], F32, tag="spart")
            for c in range(CCH):
                for j in range(NB):
                    nc.vector.reduce_sum(out=s_part[:, c, j:j + 1], in_=x_sb[c][j],
                                         axis=AX.X)
            s_red = small.tile([128, CCH], F32, tag="sred")
            nc.vector.reduce_sum(out=s_red, in_=s_part, axis=AX.X)
            s16 = small.tile([128, CCH], F16, tag="s16")
            nc.vector.tensor_copy(out=s16, in_=s_red)

            # G -> SBUF fp16
            g_sb = []
            for t in range(CCH):
                gt = consts.tile([128, 512], F16, tag=f"gsb{t}")
                nc.vector.tensor_copy(out=gt, in_=g_ps[t])
                g_sb.append(gt)

            # ---- phase 2: T = G @ Wk^T (uses G symmetry for lhsT) ----
            T_sb = []
            for t in range(CCH):
                pT = mmpsum.tile([128, 512], F32, tag="mm")
                for u in range(CCH):
                    nc.tensor.matmul(pT, lhsT=g_sb[u][:, t * 128:(t + 1) * 128],
                                     rhs=wqk_sb[u][:, 512:1024],
                                     start=(u == 0), stop=(u == CCH - 1))
                Tt = consts.tile([128, 512], F16, tag=f"T{t}")
                nc.vector.tensor_copy(out=Tt, in_=pT)
                T_sb.append(Tt)

            # u = Wqk @ s  (two [1,512] halves)
            u_row = small.tile([1, 2 * C], F32, tag="urow")
            for half in range(2):
                pu = mmpsum.tile([1, 512], F32, tag="mm")
                for c in range(CCH):
                    nc.tensor.matmul(pu, lhsT=s16[:, c:c + 1],
                                     rhs=wqk_sb[c][:, half * 512:(half + 1) * 512],
                                     start=(c == 0), stop=(c == CCH - 1))
                nc.vector.tensor_copy(out=u_row[:, half * 512:(half + 1) * 512], in_=pu)

            # w = u_q + N * b_q ; fp16 rows for rank-1 correction matmuls
            corr = small.tile([1, 2 * C], F16, tag="corr")   # [w_q | u_k]
            tmp = small.tile([1, C], F32, tag="tmpw")
            nc.vector.tensor_scalar_mul(tmp, bqk_row[:, 0:C], float(N))
            nc.vector.tensor_add(out=corr[:, 0:C], in0=u_row[:, 0:C], in1=tmp)
            nc.vector.tensor_copy(out=corr[:, C:2 * C], in_=u_row[:, C:2 * C])
            b16 = small.tile([1, 2 * C], F16, tag="b16")
            nc.vector.tensor_copy(out=b16, in_=bqk_row)

            # ---- phase 3: S accumulation (corrections + Wq @ T) ----
            s_psum = spsum.tile([D, HEADS, D], F32, tag="s")
            for h in range(HEADS):
                hs = slice(h * D, (h + 1) * D)
                # (u_q + N b_q)_h (x) bk_h
                nc.tensor.matmul(s_psum[:, h, :], lhsT=corr[:, hs],
                                 rhs=b16[:, C + h * D:C + (h + 1) * D],
                                 start=(h == 0), stop=False, skip_group_check=True)
                # bq_h (x) (Wk s)_h
                nc.tensor.matmul(s_psum[:, h, :], lhsT=b16[:, hs],
                                 rhs=corr[:, C + h * D:C + (h + 1) * D],
                                 start=False, stop=False, skip_group_check=True)
                for c in range(CCH):
                    nc.tensor.matmul(s_psum[:, h, :],
                                     lhsT=wqk_sb[c][:, hs],
                                     rhs=T_sb[c][:, hs],
                                     start=False,
                                     stop=(c == CCH - 1),
                                     skip_group_check=True)

            # ---- phase 4: v channel-major (fills PE while softmax runs) ----
            v_sb = []
            for o in range(CCH):
                t = consts.tile([128, N], F16, tag=f"v{o}")
                v_sb.append(t)
            for o in range(CCH):
                for n in range(NB):
                    pv = mmpsum.tile([128, 512], F32, tag="mm")
                    for c in range(CCH):
                        nc.tensor.matmul(pv, lhsT=wv_sb[:, c, o * 128:(o + 1) * 128],
                                         rhs=x_sb[c][n],
                                         start=(c == 0), stop=(c == CCH - 1))
                    nc.scalar.activation(v_sb[o][:, n * 512:(n + 1) * 512], pv,
                                         AF.Identity, bias=bv_sb[:, o:o + 1], scale=1.0)

            # ---- phase 5: softmax over dk ----
            mx = small.tile([D, HEADS], F32, tag="mx")
            nc.vector.reduce_max(out=mx, in_=s_psum, axis=AX.X)
            nmx = small.tile([D, HEADS], F32, tag="nmx")
            nc.vector.tensor_scalar_mul(nmx, mx, -SCALE)
            e_sb = small.tile([D, HEADS, D], F32, tag="e")
            for h in range(HEADS):
                nc.scalar.activation(e_sb[:, h, :], s_psum[:, h, :], AF.Exp,
                                     bias=nmx[:, h:h + 1], scale=SCALE)
            sm = small.tile([D, HEADS], F32, tag="sm")
            nc.vector.reduce_sum(out=sm, in_=e_sb, axis=AX.X)
            rs = small.tile([D, HEADS], F32, tag="rs")
            nc.vector.reciprocal(rs, sm)
            attn32 = small.tile([D, HEADS, D], F32, tag="attn32")
            attn16 = small.tile([D, HEADS, D], F16, tag="attn16")
            for h in range(HEADS):
                nc.vector.tensor_scalar_mul(attn32[:, h, :], e_sb[:, h, :], rs[:, h:h + 1])
            nc.vector.tensor_copy(out=attn16, in_=attn32)

            # attn_mean over heads
            am_sb = small.tile([D, D], F32, tag="am")
            attn_hview = attn32.rearrange("p h d -> p d h")
            nc.vector.reduce_sum(out=am_sb, in_=attn_hview, axis=AX.X)
            nc.vector.tensor_scalar_mul(am_sb, am_sb, 1.0 / HEADS)
            nc.sync.dma_start(out=am_d[:, :], in_=am_sb)

            # ---- phase 6: attn^T block-diag pack (2 heads per [128,128] lhsT) ----
            avl = []
            for g in range(CCH):
                t = consts.tile([128, 128], F16, tag=f"avl{g}")
                nc.vector.memset(t, 0.0)
                avl.append(t)
            for g in range(CCH):
                pt = mmpsum.tile([128, 128], F16, tag="mm")
                for r in range(2):
                    h = 2 * g + r
                    nc.tensor.transpose(pt[r * D:(r + 1) * D, r * D:(r + 1) * D],
                                        attn16[:, h, :], ident[0:D, 0:D])
                for r in range(2):
                    nc.vector.tensor_copy(out=avl[g][r * D:(r + 1) * D, r * D:(r + 1) * D],
                                          in_=pt[r * D:(r + 1) * D, r * D:(r + 1) * D])

            # ---- phase 7: av = attn @ v ----
            av_sb = []
            for g in range(CCH):
                t = consts.tile([128, N], F16, tag=f"av{g}")
                av_sb.append(t)
            for g in range(CCH):
                for n in range(NB):
                    pav = mmpsum.tile([128, 512], F32, tag="mm")
                    nc.tensor.matmul(pav, lhsT=avl[g], rhs=v_sb[g][:, n * 512:(n + 1) * 512],
                                     start=True, stop=True)
                    nc.vector.tensor_copy(out=av_sb[g][:, n * 512:(n + 1) * 512], in_=pav)

            # ---- phase 8: proj ----
            for n in range(NB):
                for o in range(CCH):
                    py = mmpsum.tile([128, 512], F32, tag="mm")
                    for c in range(CCH):
                        nc.tensor.matmul(py, lhsT=wp_sb[:, c, o * 128:(o + 1) * 128],
                                         rhs=av_sb[c][:, n * 512:(n + 1) * 512],
                                         start=(c == 0), stop=(c == CCH - 1))
                    y = ypool.tile([128, 512], F32, tag="y")
                    nc.scalar.activation(y, py, AF.Identity, bias=bp_sb[:, o:o + 1], scale=1.0)
                    nc.sync.dma_start(out=out_d[o * 128:(o + 1) * 128, n * 512:(n + 1) * 512],
                                      in_=y)

    nc.compile()
    return nc


_NC_CACHE = None


def _get_nc():
    global _NC_CACHE
    if _NC_CACHE is None:
        _NC_CACHE = build_nc()
    return _NC_CACHE


def make_in_maps(x, qkv_w, qkv_b, proj_w, proj_b):
    x = np.asarray(x, dtype=np.float32)
    qkv_w = np.asarray(qkv_w, dtype=np.float32)
    qkv_b = np.asarray(qkv_b, dtype=np.float32)
    proj_w = np.asarray(proj_w, dtype=np.float32)
    proj_b = np.asarray(proj_b, dtype=np.float32)
    B = x.shape[0]
    xm = x.reshape(B, C, N).astype(np.float16)
    wqk = np.ascontiguousarray(qkv_w[:2 * C].T).astype(np.float16)
    wv = np.ascontiguousarray(qkv_w[2 * C:].T).astype(np.float16)
    wp = np.ascontiguousarray(proj_w.T).astype(np.float16)
    bqk = np.ascontiguousarray(qkv_b[:2 * C])
    bv = np.ascontiguousarray(qkv_b[2 * C:])
    bp = np.ascontiguousarray(proj_b)
    return [
        {"x": xm[b], "wqk": wqk, "wv": wv, "wp": wp, "bqk": bqk, "bv": bv, "bp": bp}
        for b in range(B)
    ]


def kernel(x, qkv_w, qkv_b, proj_w, proj_b):
    B, _, H, W = np.asarray(x).shape
    in_maps = make_in_maps(x, qkv_w, qkv_b, proj_w, proj_b)
    nc = _get_nc()
    res = run_bass_kernel_spmd(nc, in_maps, core_ids=list(range(B)))
    out = np.stack([res.results[b]["out"] for b in range(B)]).reshape(B, C, H, W)
    am = np.stack([res.results[b]["attn_mean"] for b in range(B)])
    return out.astype(np.float32), am.astype(np.float32)


if __name__ == "__main__":
    nc = build_nc()
    print("built ok")


# revision 8
# speedup vs baseline: 1.1176x; 1.0869x over previous
"""Trainium2 Bass kernel for AttentionModule (channel attention over 64x64 images).

Computation (per batch b):
  qkv = qkv_w @ x + qkv_b            (1x1 conv, x viewed as [C, N=H*W])
  q,k,v: [heads=8, d=64, N]
  S_h   = q_h @ k_h^T * d^-0.5       ([64, 64] per head -- contraction over N!)
  attn  = softmax(S, axis=-1)
  out   = proj_w @ (attn @ v) + proj_b
  returns (out [B,C,H,W], attn.mean(heads) [B,64,64])

Sharding: data-parallel over batch, 1 batch per NeuronCore, weights replicated.

Per-core dataflow (fp16 matmul inputs, fp32 PSUM accumulation):
  - Scores via the Gram identity: S_h = Wq_h G Wk_h^T + rank-1 bias terms,
    with G = x x^T accumulated from DMA-transposed x tiles (no PE transposes).
  - attn@v and proj fused: out = (Wp A) V with MT = A^T Wp^T computed directly
    from attn tiles (block-diagonal A never materialized).
"""

import sys

for _p in ("/root/.axon_site/_ro/trn_rl_repo", "/opt/trn_rl_repo"):
    if _p not in sys.path:
        sys.path.append(_p)

import numpy as np

import concourse.bacc as bacc
import concourse.bass as bass
import concourse.mybir as mybir
import concourse.tile as tile
from concourse.bass_utils import run_bass_kernel_spmd

F16 = mybir.dt.float16
F32 = mybir.dt.float32
AF = mybir.ActivationFunctionType
AX = mybir.AxisListType

C = 512          # channels
N = 4096         # pixels (64*64)
HEADS = 8
D = 64           # head dim
CCH = C // 128   # channel chunks of 128 partitions
NB = N // 512    # x column blocks (512 wide)
NT = N // 128    # pixel chunks of 128
SCALE = float(D) ** -0.5


def build_nc():
    nc = bacc.Bacc("TRN2", target_bir_lowering=False, debug=False)

    x_d = nc.dram_tensor("x", [C, N], F16, kind="ExternalInput")
    wqk_d = nc.dram_tensor("wqk", [C, 2 * C], F16, kind="ExternalInput")  # qkv_w[:2C].T
    wv_d = nc.dram_tensor("wv", [C, C], F16, kind="ExternalInput")        # qkv_w[2C:].T
    wp_d = nc.dram_tensor("wp", [D, HEADS, C], F16, kind="ExternalInput")  # proj_w.T head-major
    bqk_d = nc.dram_tensor("bqk", [2 * C], F32, kind="ExternalInput")
    bv_d = nc.dram_tensor("bv", [C], F32, kind="ExternalInput")
    bp_d = nc.dram_tensor("bp", [C], F32, kind="ExternalInput")
    out_d = nc.dram_tensor("out", [C, N], F32, kind="ExternalOutput")
    am_d = nc.dram_tensor("attn_mean", [D, D], F32, kind="ExternalOutput")

    with tile.TileContext(nc) as tc:
        with (
            tc.tile_pool(name="consts", bufs=1) as consts,
            tc.tile_pool(name="small", bufs=1) as small,
            tc.tile_pool(name="ypool", bufs=4) as ypool,
            tc.tile_pool(name="mmpsum", bufs=3, space="PSUM") as mmpsum,
            tc.tile_pool(name="gpsum", bufs=1, space="PSUM") as gpsum,
            tc.tile_pool(name="spsum", bufs=1, space="PSUM") as spsum,
        ):
            # ---- input DMAs ----
            # x^T tiles via DMA transpose (these gate G, so issue them first)
            xT_sb = []
            for n in range(NT):
                t = consts.tile([128, 512], F16, tag=f"xt{n}")
                nc.sync.dma_start_transpose(t, x_d[:, n * 128:(n + 1) * 128])
                xT_sb.append(t)

            wqk_sb = []
            for c in range(CCH):
                t = consts.tile([128, 2 * C], F16, tag=f"wqk{c}")
                nc.sync.dma_start(out=t, in_=wqk_d[c * 128:(c + 1) * 128, :])
                wqk_sb.append(t)
            wv_sb = consts.tile([128, CCH, C], F16, tag="wv")
            nc.sync.dma_start(out=wv_sb, in_=wv_d.rearrange("(cc p) o -> p cc o", p=128))
            wp_sb = consts.tile([D, HEADS, C], F16, tag="wp")
            nc.sync.dma_start(out=wp_sb, in_=wp_d[:, :, :])

            x_sb = [[None] * NB for _ in range(CCH)]
            for j in range(NB):
                for c in range(CCH):
                    t = consts.tile([128, 512], F16, tag=f"x{c}_{j}")
                    nc.sync.dma_start(out=t, in_=x_d[c * 128:(c + 1) * 128,
                                                     j * 512:(j + 1) * 512])
                    x_sb[c][j] = t

            bqk_row = small.tile([1, 2 * C], F32, tag="bqkrow")
            nc.sync.dma_start(out=bqk_row, in_=bqk_d[None, :])
            bv_sb = consts.tile([128, CCH], F32, tag="bv")
            nc.sync.dma_start(out=bv_sb, in_=bv_d.rearrange("(o p) -> p o", p=128))
            bp_sb = consts.tile([128, CCH], F32, tag="bp")
            nc.sync.dma_start(out=bp_sb, in_=bp_d.rearrange("(o p) -> p o", p=128))

            # ---- phase 1: G = x x^T accumulated over pixel chunks ----
            g_ps = [gpsum.tile([128, 512], F32, tag=f"g{t}", name=f"g{t}")
                    for t in range(CCH)]
            for n in range(NT):
                xt = xT_sb[n]
                for t in range(CCH):
                    nc.tensor.matmul(g_ps[t], lhsT=xt[:, t * 128:(t + 1) * 128],
                                     rhs=xt[:, :],
                                     start=(n == 0), stop=(n == NT - 1))

            # row sums s = x @ 1 (DVE; overlaps later PE phases)
            s_part = small.tile([128, CCH, NB], F32, tag="spart")
            for c in range(CCH):
                for j in range(NB):
                    nc.vector.reduce_sum(out=s_part[:, c, j:j + 1], in_=x_sb[c][j],
                                         axis=AX.X)
            s_red = small.tile([128, CCH], F32, tag="sred")
            nc.vector.reduce_sum(out=s_red, in_=s_part, axis=AX.X)
            s16 = small.tile([128, CCH], F16, tag="s16")
            nc.vector.tensor_copy(out=s16, in_=s_red)

            # G -> SBUF fp16 (split DVE/ACT)
            g_sb = []
            for t in range(CCH):
                gt = consts.tile([128, 512], F16, tag=f"gsb{t}")
                if t % 2 == 0:
                    nc.vector.tensor_copy(out=gt, in_=g_ps[t])
                else:
                    nc.scalar.activation(gt, g_ps[t], AF.Identity, bias=0.0, scale=1.0)
                g_sb.append(gt)

            # ---- phase 2: T = G @ Wk^T (uses G symmetry for lhsT) ----
            T_sb = []
            for t in range(CCH):
                pT = mmpsum.tile([128, 512], F32, tag="mm")
                for u in range(CCH):
                    nc.tensor.matmul(pT, lhsT=g_sb[u][:, t * 128:(t + 1) * 128],
                                     rhs=wqk_sb[u][:, 512:1024],
                                     start=(u == 0), stop=(u == CCH - 1))
                Tt = consts.tile([128, 512], F16, tag=f"T{t}")
                nc.vector.tensor_copy(out=Tt, in_=pT)
                T_sb.append(Tt)

            # u = Wqk @ s  (two [1,512] halves)
            u_row = small.tile([1, 2 * C], F32, tag="urow")
            for half in range(2):
                pu = mmpsum.tile([1, 512], F32, tag="mm")
                for c in range(CCH):
                    nc.tensor.matmul(pu, lhsT=s16[:, c:c + 1],
                                     rhs=wqk_sb[c][:, half * 512:(half + 1) * 512],
                                     start=(c == 0), stop=(c == CCH - 1))
                nc.vector.tensor_copy(out=u_row[:, half * 512:(half + 1) * 512], in_=pu)

            # w = u_q + N * b_q ; fp16 rows for rank-1 correction matmuls
            corr = small.tile([1, 2 * C], F16, tag="corr")   # [w_q | u_k]
            tmp = small.tile([1, C], F32, tag="tmpw")
            nc.vector.tensor_scalar_mul(tmp, bqk_row[:, 0:C], float(N))
            nc.vector.tensor_add(out=corr[:, 0:C], in0=u_row[:, 0:C], in1=tmp)
            nc.vector.tensor_copy(out=corr[:, C:2 * C], in_=u_row[:, C:2 * C])
            b16 = small.tile([1, 2 * C], F16, tag="b16")
            nc.vector.tensor_copy(out=b16, in_=bqk_row)

            # ---- phase 3: S accumulation (corrections + Wq @ T) ----
            s_psum = spsum.tile([D, HEADS, D], F32, tag="s")
            for h in range(HEADS):
                hs = slice(h * D, (h + 1) * D)
                nc.tensor.matmul(s_psum[:, h, :], lhsT=corr[:, hs],
                                 rhs=b16[:, C + h * D:C + (h + 1) * D],
                                 start=(h == 0), stop=False, skip_group_check=True)
                nc.tensor.matmul(s_psum[:, h, :], lhsT=b16[:, hs],
                                 rhs=corr[:, C + h * D:C + (h + 1) * D],
                                 start=False, stop=False, skip_group_check=True)
                for c in range(CCH):
                    nc.tensor.matmul(s_psum[:, h, :],
                                     lhsT=wqk_sb[c][:, hs],
                                     rhs=T_sb[c][:, hs],
                                     start=False,
                                     stop=(c == CCH - 1),
                                     skip_group_check=True)

            # ---- phase 4: softmax over dk (DVE/ACT; PE proceeds to v) ----
            mx = small.tile([D, HEADS], F32, tag="mx")
            nc.vector.reduce_max(out=mx, in_=s_psum, axis=AX.X)
            nmx = small.tile([D, HEADS], F32, tag="nmx")
            nc.vector.tensor_scalar_mul(nmx, mx, -SCALE)
            e_sb = small.tile([D, HEADS, D], F32, tag="e")
            for h in range(HEADS):
                nc.scalar.activation(e_sb[:, h, :], s_psum[:, h, :], AF.Exp,
                                     bias=nmx[:, h:h + 1], scale=SCALE)
            sm = small.tile([D, HEADS], F32, tag="sm")
            nc.vector.reduce_sum(out=sm, in_=e_sb, axis=AX.X)
            rs = small.tile([D, HEADS], F32, tag="rs")
            nc.vector.reciprocal(rs, sm)
            attn32 = small.tile([D, HEADS, D], F32, tag="attn32")
            attn16 = small.tile([D, HEADS, D], F16, tag="attn16")
            for h in range(HEADS):
                nc.vector.tensor_scalar_mul(attn32[:, h, :], e_sb[:, h, :], rs[:, h:h + 1])
            nc.vector.tensor_copy(out=attn16, in_=attn32)

            # attn_mean over heads
            am_sb = small.tile([D, D], F32, tag="am")
            attn_hview = attn32.rearrange("p h d -> p d h")
            nc.vector.reduce_sum(out=am_sb, in_=attn_hview, axis=AX.X)
            nc.vector.tensor_scalar_mul(am_sb, am_sb, 1.0 / HEADS)
            nc.sync.dma_start(out=am_d[:, :], in_=am_sb)

            # ---- phase 5: v channel-major ----
            v_sb = []
            for o in range(CCH):
                t = consts.tile([128, N], F16, tag=f"v{o}")
                v_sb.append(t)
            for o in range(CCH):
                for n in range(NB):
                    pv = mmpsum.tile([128, 512], F32, tag="mm")
                    for c in range(CCH):
                        nc.tensor.matmul(pv, lhsT=wv_sb[:, c, o * 128:(o + 1) * 128],
                                         rhs=x_sb[c][n],
                                         start=(c == 0), stop=(c == CCH - 1))
                    nc.scalar.activation(v_sb[o][:, n * 512:(n + 1) * 512], pv,
                                         AF.Identity, bias=bv_sb[:, o:o + 1], scale=1.0)

            # ---- phase 6: MT = (Wp A)^T per head pair, straight from attn16 ----
            # MT rows e of head h: attn_h^T @ WpT[d-rows of h] -> psum partitions
            # (h%2)*64..(h%2+1)*64 of the pair tile.
            MT_sb = []
            for g in range(CCH):
                pmt = mmpsum.tile([128, 512], F32, tag="mm")
                for r in range(2):
                    h = 2 * g + r
                    nc.tensor.matmul(pmt[r * D:(r + 1) * D, :],
                                     lhsT=attn16[:, h, :],
                                     rhs=wp_sb[:, h, :],
                                     start=True, stop=True,
                                     skip_group_check=True)
                mt = consts.tile([128, 512], F16, tag=f"mt{g}")
                nc.vector.tensor_copy(out=mt, in_=pmt)
                MT_sb.append(mt)

            # ---- phase 7: out = MT^T @ V + bp ----
            for n in range(NB):
                for o in range(CCH):
                    py = mmpsum.tile([128, 512], F32, tag="mm")
                    for g in range(CCH):
                        nc.tensor.matmul(py, lhsT=MT_sb[g][:, o * 128:(o + 1) * 128],
                                         rhs=v_sb[g][:, n * 512:(n + 1) * 512],
                                         start=(g == 0), stop=(g == CCH - 1))
                    y = ypool.tile([128, 512], F32, tag="y")
                    if o % 2 == 0:
                        nc.scalar.activation(y, py, AF.Identity, bias=bp_sb[:, o:o + 1],
                                             scale=1.0)
                    else:
                        nc.vector.tensor_scalar_add(y, py, bp_sb[:, o:o + 1])
                    nc.sync.dma_start(out=out_d[o * 128:(o + 1) * 128,
                                                n * 512:(n + 1) * 512],
                                      in_=y)

    nc.compile()
    return nc


_NC_CACHE = None


def _get_nc():
    global _NC_CACHE
    if _NC_CACHE is None:
        _NC_CACHE = build_nc()
    return _NC_CACHE


def make_in_maps(x, qkv_w, qkv_b, proj_w, proj_b):
    x = np.asarray(x, dtype=np.float32)
    qkv_w = np.asarray(qkv_w, dtype=np.float32)
    qkv_b = np.asarray(qkv_b, dtype=np.float32)
    proj_w = np.asarray(proj_w, dtype=np.float32)
    proj_b = np.asarray(proj_b, dtype=np.float32)
    B = x.shape[0]
    xm = x.reshape(B, C, N).astype(np.float16)
    wqk = np.ascontiguousarray(qkv_w[:2 * C].T).astype(np.float16)
    wv = np.ascontiguousarray(qkv_w[2 * C:].T).astype(np.float16)
    wp = np.ascontiguousarray(
        proj_w.T.reshape(HEADS, D, C).transpose(1, 0, 2)).astype(np.float16)
    bqk = np.ascontiguousarray(qkv_b[:2 * C])
    bv = np.ascontiguousarray(qkv_b[2 * C:])
    bp = np.ascontiguousarray(proj_b)
    return [
        {"x": xm[b], "wqk": wqk, "wv": wv, "wp": wp, "bqk": bqk, "bv": bv, "bp": bp}
        for b in range(B)
    ]


def kernel(x, qkv_w, qkv_b, proj_w, proj_b):
    B, _, H, W = np.asarray(x).shape
    in_maps = make_in_maps(x, qkv_w, qkv_b, proj_w, proj_b)
    nc = _get_nc()
    res = run_bass_kernel_spmd(nc, in_maps, core_ids=list(range(B)))
    out = np.stack([res.results[b]["out"] for b in range(B)]).reshape(B, C, H, W)
    am = np.stack([res.results[b]["attn_mean"] for b in range(B)])
    return out.astype(np.float32), am.astype(np.float32)


if __name__ == "__main__":
    nc = build_nc()
    print("built ok")


# revision 11
# speedup vs baseline: 1.1309x; 1.0118x over previous
"""Trainium2 Bass kernel for AttentionModule (channel attention over 64x64 images).

Computation (per batch b):
  qkv = qkv_w @ x + qkv_b            (1x1 conv, x viewed as [C, N=H*W])
  q,k,v: [heads=8, d=64, N]
  S_h   = q_h @ k_h^T * d^-0.5       ([64, 64] per head -- contraction over N!)
  attn  = softmax(S, axis=-1)
  out   = proj_w @ (attn @ v) + proj_b
  returns (out [B,C,H,W], attn.mean(heads) [B,64,64])

Sharding: data-parallel over batch, 1 batch per NeuronCore, weights replicated.

Per-core dataflow (fp16 matmul inputs, fp32 PSUM accumulation):
  - Scores via the Gram identity: S_h = Wq_h G Wk_h^T + rank-1 bias terms,
    with G = x x^T accumulated from DMA-transposed x tiles (no PE transposes).
  - attn@v and proj fused: out = (Wp A) V with MT = A^T Wp^T computed directly
    from attn tiles (block-diagonal A never materialized).
"""

import os
import sys

os.environ.setdefault("NEURON_RT_RESET_CORES", "1")

for _p in ("/root/.axon_site/_ro/trn_rl_repo", "/opt/trn_rl_repo"):
    if _p not in sys.path:
        sys.path.append(_p)

import numpy as np

import concourse.bacc as bacc
import concourse.bass as bass
import concourse.mybir as mybir
import concourse.tile as tile
from concourse.bass_utils import run_bass_kernel_spmd
from concourse.masks import make_identity

F16 = mybir.dt.float16
F32 = mybir.dt.float32
AF = mybir.ActivationFunctionType
AX = mybir.AxisListType

C = 512          # channels
N = 4096         # pixels (64*64)
HEADS = 8
D = 64           # head dim
CCH = C // 128   # channel chunks of 128 partitions
NB = N // 512    # x column blocks (512 wide)
NT = N // 128    # pixel chunks of 128
SCALE = float(D) ** -0.5


def build_nc():
    nc = bacc.Bacc("TRN2", target_bir_lowering=False, debug=False)

    x_d = nc.dram_tensor("x", [C, N], F16, kind="ExternalInput")
    wqk_d = nc.dram_tensor("wqk", [C, 2 * C], F16, kind="ExternalInput")  # qkv_w[:2C].T
    wv_d = nc.dram_tensor("wv", [C, C], F16, kind="ExternalInput")        # qkv_w[2C:].T
    wp_d = nc.dram_tensor("wp", [D, HEADS, C], F16, kind="ExternalInput")  # proj_w.T head-major
    bqk_d = nc.dram_tensor("bqk", [2 * C], F32, kind="ExternalInput")
    bv_d = nc.dram_tensor("bv", [C], F32, kind="ExternalInput")
    bp_d = nc.dram_tensor("bp", [C], F32, kind="ExternalInput")
    out_d = nc.dram_tensor("out", [C, N], F16, kind="ExternalOutput")
    am_d = nc.dram_tensor("attn_mean", [D, D], F32, kind="ExternalOutput")

    with tile.TileContext(nc) as tc:
        with (
            tc.tile_pool(name="consts", bufs=1) as consts,
            tc.tile_pool(name="small", bufs=1) as small,
            tc.tile_pool(name="ypool", bufs=4) as ypool,
            tc.tile_pool(name="mmpsum", bufs=3, space="PSUM") as mmpsum,
            tc.tile_pool(name="gpsum", bufs=1, space="PSUM") as gpsum,
            tc.tile_pool(name="spsum", bufs=1, space="PSUM") as spsum,
        ):
            # ---- input DMAs ----
            # x in half-row tiles; first half of every channel chunk lands first
            # so PE transposes can start early. Split across both HWDGE queues.
            HN = N // 2
            x_sb = [[None, None] for _ in range(CCH)]
            for half in range(2):
                for c in range(CCH):
                    t = consts.tile([128, HN], F16, tag=f"x{c}_{half}")
                    eng = nc.sync if c % 2 == 0 else nc.scalar
                    eng.dma_start(out=t, in_=x_d[c * 128:(c + 1) * 128,
                                                 half * HN:(half + 1) * HN])
                    x_sb[c][half] = t

            wqk_all = consts.tile([128, CCH, 2 * C], F16, tag="wqkall")
            nc.scalar.dma_start(out=wqk_all,
                                in_=wqk_d.rearrange("(cc p) o -> p cc o", p=128))
            wqk_sb = [wqk_all[:, c, :] for c in range(CCH)]
            wv_sb = consts.tile([128, CCH, C], F16, tag="wv")
            nc.sync.dma_start(out=wv_sb, in_=wv_d.rearrange("(cc p) o -> p cc o", p=128))
            wp_sb = consts.tile([D, HEADS, C], F16, tag="wp")
            nc.sync.dma_start(out=wp_sb, in_=wp_d[:, :, :])

            ident = consts.tile([128, 128], F16, tag="ident")
            make_identity(nc, ident)

            bqk_row = small.tile([1, 2 * C], F32, tag="bqkrow")
            nc.gpsimd.dma_start(out=bqk_row, in_=bqk_d[None, :])
            bv_sb = consts.tile([128, CCH], F32, tag="bv")
            nc.gpsimd.dma_start(out=bv_sb, in_=bv_d.rearrange("(o p) -> p o", p=128))
            bp_sb = consts.tile([128, CCH], F32, tag="bp")
            nc.gpsimd.dma_start(out=bp_sb, in_=bp_d.rearrange("(o p) -> p o", p=128))

            # ---- phase 1: transpose x on PE and accumulate G = x x^T ----
            g_ps = [gpsum.tile([128, 512], F32, tag=f"g{t}", name=f"g{t}")
                    for t in range(CCH)]
            xT_sb = []
            for n in range(NT):
                half, s = divmod(n, NT // 2)
                pxt = mmpsum.tile([128, 512], F16, tag="mm")
                for c in range(CCH):
                    nc.tensor.transpose(pxt[:, c * 128:(c + 1) * 128],
                                        x_sb[c][half][:, s * 128:(s + 1) * 128], ident)
                xt = consts.tile([128, 512], F16, tag=f"xt{n}")
                if n % 2 == 0:
                    nc.vector.tensor_copy(out=xt, in_=pxt)
                else:
                    nc.scalar.activation(xt, pxt, AF.Identity, bias=0.0, scale=1.0)
                xT_sb.append(xt)
                for t in range(CCH):
                    nc.tensor.matmul(g_ps[t], lhsT=xt[:, t * 128:(t + 1) * 128],
                                     rhs=xt[:, :],
                                     start=(n == 0), stop=(n == NT - 1))

            # row sums s = x @ 1 (DVE; overlaps later PE phases)
            s_part = small.tile([128, CCH, 2], F32, tag="spart")
            for c in range(CCH):
                for half in range(2):
                    nc.vector.reduce_sum(out=s_part[:, c, half:half + 1],
                                         in_=x_sb[c][half], axis=AX.X)
            s_red = small.tile([128, CCH], F32, tag="sred")
            nc.vector.reduce_sum(out=s_red, in_=s_part, axis=AX.X)
            s16 = small.tile([128, CCH], F16, tag="s16")
            nc.vector.tensor_copy(out=s16, in_=s_red)

            # G -> SBUF fp16 (split DVE/ACT)
            g_sb = []
            for t in range(CCH):
                gt = consts.tile([128, 512], F16, tag=f"gsb{t}")
                if t % 2 == 0:
                    nc.vector.tensor_copy(out=gt, in_=g_ps[t])
                else:
                    nc.scalar.activation(gt, g_ps[t], AF.Identity, bias=0.0, scale=1.0)
                g_sb.append(gt)

            # ---- phase 2: T = G @ Wk^T (uses G symmetry for lhsT) ----
            T_sb = []
            for t in range(CCH):
                pT = mmpsum.tile([128, 512], F32, tag="mm")
                for u in range(CCH):
                    nc.tensor.matmul(pT, lhsT=g_sb[u][:, t * 128:(t + 1) * 128],
                                     rhs=wqk_sb[u][:, 512:1024],
                                     start=(u == 0), stop=(u == CCH - 1))
                Tt = consts.tile([128, 512], F16, tag=f"T{t}")
                nc.vector.tensor_copy(out=Tt, in_=pT)
                T_sb.append(Tt)

            # u = Wqk @ s  (two [1,512] halves)
            u_row = small.tile([1, 2 * C], F32, tag="urow")
            for half in range(2):
                pu = mmpsum.tile([1, 512], F32, tag="mm")
                for c in range(CCH):
                    nc.tensor.matmul(pu, lhsT=s16[:, c:c + 1],
                                     rhs=wqk_sb[c][:, half * 512:(half + 1) * 512],
                                     start=(c == 0), stop=(c == CCH - 1))
                nc.vector.tensor_copy(out=u_row[:, half * 512:(half + 1) * 512], in_=pu)

            # w = u_q + N * b_q ; fp16 rows for rank-1 correction matmuls
            corr = small.tile([1, 2 * C], F16, tag="corr")   # [w_q | u_k]
            tmp = small.tile([1, C], F32, tag="tmpw")
            nc.vector.tensor_scalar_mul(tmp, bqk_row[:, 0:C], float(N))
            nc.vector.tensor_add(out=corr[:, 0:C], in0=u_row[:, 0:C], in1=tmp)
            nc.vector.tensor_copy(out=corr[:, C:2 * C], in_=u_row[:, C:2 * C])
            b16 = small.tile([1, 2 * C], F16, tag="b16")
            nc.vector.tensor_copy(out=b16, in_=bqk_row)

            # ---- phase 3: S accumulation (corrections + Wq @ T) ----
            s_psum = spsum.tile([D, HEADS, D], F32, tag="s")
            for h in range(HEADS):
                hs = slice(h * D, (h + 1) * D)
                nc.tensor.matmul(s_psum[:, h, :], lhsT=corr[:, hs],
                                 rhs=b16[:, C + h * D:C + (h + 1) * D],
                                 start=(h == 0), stop=False, skip_group_check=True)
                nc.tensor.matmul(s_psum[:, h, :], lhsT=b16[:, hs],
                                 rhs=corr[:, C + h * D:C + (h + 1) * D],
                                 start=False, stop=False, skip_group_check=True)
                for c in range(CCH):
                    nc.tensor.matmul(s_psum[:, h, :],
                                     lhsT=wqk_sb[c][:, hs],
                                     rhs=T_sb[c][:, hs],
                                     start=False,
                                     stop=(c == CCH - 1),
                                     skip_group_check=True)

            # ---- phase 4: softmax over dk (DVE/ACT; PE proceeds to v) ----
            mx = small.tile([D, HEADS], F32, tag="mx")
            nc.vector.reduce_max(out=mx, in_=s_psum, axis=AX.X)
            nmx = small.tile([D, HEADS], F32, tag="nmx")
            nc.vector.tensor_scalar_mul(nmx, mx, -SCALE)
            e_sb = small.tile([D, HEADS, D], F32, tag="e")
            for h in range(HEADS):
                nc.scalar.activation(e_sb[:, h, :], s_psum[:, h, :], AF.Exp,
                                     bias=nmx[:, h:h + 1], scale=SCALE)
            sm = small.tile([D, HEADS], F32, tag="sm")
            nc.vector.reduce_sum(out=sm, in_=e_sb, axis=AX.X)
            rs = small.tile([D, HEADS], F32, tag="rs")
            nc.vector.reciprocal(rs, sm)
            attn32 = small.tile([D, HEADS, D], F32, tag="attn32")
            attn16 = small.tile([D, HEADS, D], F16, tag="attn16")
            for h in range(HEADS):
                nc.vector.tensor_scalar_mul(attn32[:, h, :], e_sb[:, h, :], rs[:, h:h + 1])
            nc.vector.tensor_copy(out=attn16, in_=attn32)

            # attn_mean over heads
            am_sb = small.tile([D, D], F32, tag="am")
            attn_hview = attn32.rearrange("p h d -> p d h")
            nc.vector.reduce_sum(out=am_sb, in_=attn_hview, axis=AX.X)
            nc.vector.tensor_scalar_mul(am_sb, am_sb, 1.0 / HEADS)
            nc.sync.dma_start(out=am_d[:, :], in_=am_sb)

            # ---- phase 5: v channel-major ----
            v_sb = []
            for o in range(CCH):
                t = consts.tile([128, N], F16, tag=f"v{o}")
                v_sb.append(t)
            for o in range(CCH):
                for n in range(NB):
                    pv = mmpsum.tile([128, 512], F32, tag="mm")
                    for c in range(CCH):
                        nc.tensor.matmul(pv, lhsT=wv_sb[:, c, o * 128:(o + 1) * 128],
                                         rhs=x_sb[c][n // 4][:, (n % 4) * 512:(n % 4 + 1) * 512],
                                         start=(c == 0), stop=(c == CCH - 1))
                    nc.scalar.activation(v_sb[o][:, n * 512:(n + 1) * 512], pv,
                                         AF.Identity, bias=bv_sb[:, o:o + 1], scale=1.0)

            # ---- phase 6: MT = (Wp A)^T per head pair, straight from attn16 ----
            # MT rows e of head h: attn_h^T @ WpT[d-rows of h] -> psum partitions
            # (h%2)*64..(h%2+1)*64 of the pair tile.
            MT_sb = []
            for g in range(CCH):
                pmt = mmpsum.tile([128, 512], F32, tag="mm")
                for r in range(2):
                    h = 2 * g + r
                    nc.tensor.matmul(pmt[r * D:(r + 1) * D, :],
                                     lhsT=attn16[:, h, :],
                                     rhs=wp_sb[:, h, :],
                                     start=True, stop=True,
                                     skip_group_check=True)
                mt = consts.tile([128, 512], F16, tag=f"mt{g}")
                nc.vector.tensor_copy(out=mt, in_=pmt)
                MT_sb.append(mt)

            # ---- phase 7: out = MT^T @ V + bp ----
            for n in range(NB):
                y = ypool.tile([128, CCH, 512], F16, tag="y")
                for o in range(CCH):
                    py = mmpsum.tile([128, 512], F32, tag="mm")
                    for g in range(CCH):
                        nc.tensor.matmul(py, lhsT=MT_sb[g][:, o * 128:(o + 1) * 128],
                                         rhs=v_sb[g][:, n * 512:(n + 1) * 512],
                                         start=(g == 0), stop=(g == CCH - 1))
                    if o % 2 == 0:
                        nc.scalar.activation(y[:, o, :], py, AF.Identity,
                                             bias=bp_sb[:, o:o + 1], scale=1.0)
                    else:
                        nc.vector.tensor_scalar_add(y[:, o, :], py, bp_sb[:, o:o + 1])
                nc.sync.dma_start(
                    out=out_d[:, n * 512:(n + 1) * 512].rearrange("(o p) n -> p o n",
                                                                  p=128),
                    in_=y)

    nc.compile()
    return nc




# ---- cached SPMD runner (avoids per-call jax retrace + host-side zero upload) ----
_RUN_CACHE = {}


def _make_runner(nc, n_cores):
    import jax
    import jax.numpy as jnp
    import numpy as _np
    from jax.experimental.shard_map import shard_map
    from jax.sharding import Mesh, NamedSharding, PartitionSpec
    from concourse import bass2jax, mybir as _mybir

    bass2jax.install_neuronx_cc_hook()
    partition_name = nc.partition_id_tensor.name if nc.partition_id_tensor else None
    in_names, out_names, out_avals = [], [], []
    for alloc in nc.m.functions[0].allocations:
        if not isinstance(alloc, _mybir.MemoryLocationSet):
            continue
        name = alloc.memorylocations[0].name
        if alloc.kind == "ExternalInput":
            if name != partition_name:
                in_names.append(name)
        elif alloc.kind == "ExternalOutput":
            out_names.append(name)
            out_avals.append(jax.core.ShapedArray(tuple(alloc.tensor_shape),
                                                  _mybir.dt.np(alloc.dtype)))
    n_params = len(in_names)
    n_outs = len(out_names)
    all_names = list(in_names) + list(out_names)
    if partition_name is not None:
        all_names.append(partition_name)

    def _body(*args):
        operands = list(args)
        if partition_name is not None:
            operands.append(bass2jax.partition_id_tensor())
        outs = bass2jax._bass_exec_p.bind(
            *operands,
            out_avals=tuple(out_avals),
            in_names=tuple(all_names),
            out_names=tuple(out_names),
            lowering_input_output_aliases=(),
            sim_require_finite=True,
            sim_require_nnan=True,
            nc=nc,
        )
        return tuple(outs)

    devices = jax.devices()[:n_cores]
    mesh = Mesh(_np.asarray(devices), ("core",))
    in_specs = (PartitionSpec("core"),) * (n_params + n_outs)
    out_specs = (PartitionSpec("core"),) * n_outs
    donate = tuple(range(n_params, n_params + n_outs))
    sharded = jax.jit(
        shard_map(_body, mesh=mesh, in_specs=in_specs, out_specs=out_specs,
                  check_rep=False),
        donate_argnums=donate, keep_unused=True)

    zero_shardings = [NamedSharding(mesh, PartitionSpec("core"))] * n_outs

    def _zeros():
        return [
            jax.jit(lambda a=a: jnp.zeros((n_cores * a.shape[0], *a.shape[1:]),
                                          a.dtype),
                    out_shardings=s)()
            for a, s in zip(out_avals, zero_shardings)
        ]

    def run(in_maps):
        concat_in = [
            np.concatenate([np.asarray(in_maps[c][name]) for c in range(n_cores)],
                           axis=0)
            for name in in_names
        ]
        out_arrs = sharded(*concat_in, *_zeros())
        return [
            {name: np.asarray(out_arrs[i]).reshape(n_cores, *out_avals[i].shape)[c]
             for i, name in enumerate(out_names)}
            for c in range(n_cores)
        ]

    return run


def run_spmd_cached(nc, in_maps):
    key = id(nc)
    if key not in _RUN_CACHE:
        _RUN_CACHE[key] = _make_runner(nc, len(in_maps))
    return _RUN_CACHE[key](in_maps)


_NC_CACHE = None


def _get_nc():
    global _NC_CACHE
    if _NC_CACHE is None:
        _NC_CACHE = build_nc()
    return _NC_CACHE


def make_in_maps(x, qkv_w, qkv_b, proj_w, proj_b):
    x = np.asarray(x, dtype=np.float32)
    qkv_w = np.asarray(qkv_w, dtype=np.float32)
    qkv_b = np.asarray(qkv_b, dtype=np.float32)
    proj_w = np.asarray(proj_w, dtype=np.float32)
    proj_b = np.asarray(proj_b, dtype=np.float32)
    B = x.shape[0]
    xm = x.reshape(B, C, N).astype(np.float16)
    wqk = np.ascontiguousarray(qkv_w[:2 * C].T).astype(np.float16)
    wv = np.ascontiguousarray(qkv_w[2 * C:].T).astype(np.float16)
    wp = np.ascontiguousarray(
        proj_w.T.reshape(HEADS, D, C).transpose(1, 0, 2)).astype(np.float16)
    bqk = np.ascontiguousarray(qkv_b[:2 * C])
    bv = np.ascontiguousarray(qkv_b[2 * C:])
    bp = np.ascontiguousarray(proj_b)
    return [
        {"x": xm[b], "wqk": wqk, "wv": wv, "wp": wp, "bqk": bqk, "bv": bv, "bp": bp}
        for b in range(B)
    ]


def kernel(x, qkv_w, qkv_b, proj_w, proj_b):
    B, _, H, W = np.asarray(x).shape
    in_maps = make_in_maps(x, qkv_w, qkv_b, proj_w, proj_b)
    nc = _get_nc()
    results = run_spmd_cached(nc, in_maps)
    out = np.stack([results[b]["out"] for b in range(B)]).reshape(B, C, H, W)
    am = np.stack([results[b]["attn_mean"] for b in range(B)])
    return out.astype(np.float32), am.astype(np.float32)


if __name__ == "__main__":
    nc = build_nc()
    print("built ok")
